# revision 5
# baseline (speedup 1.0000x reference)
"""Bass/Tile kernel for nn_AligningModel on 8 Trainium2 NeuronCores.

Data parallel: 32 samples sharded 4-per-core across 8 cores; all params
replicated.  Inside each core the model runs feature-major ([C, T] layout)
with bf16 matmul operands and fp32 PSUM accumulation:

  mel GLU encoder (4 layers)  -> mel_enc   [384, T]
  phoneme GLU encoder (4)     -> ph_enc    [384, 257]
  -L2^2 attention softmax     -> context   [384, T]   (the |mel|^2 term
      cancels inside softmax, so scores are 2*mel.ph - |ph|^2 - penalty)
  dec GLU (2 layers, 768ch)   -> dec       [768, T]
  logits twice (token-major for log_softmax, feature-major for mel decoder)
  mel GLU decoder (2 layers, 128ch) -> tanh mel preds

Convolutions (k=3, SAME) are shifted matmuls accumulated in PSUM.
"""

import numpy as np
import ml_dtypes

BF = ml_dtypes.bfloat16

B_FULL = 32
T_MEL = 2000
T_PHON = 256
E = 384
V = 256
MEL_DIMS = 80
DEC_H = 128
N_CORES = 8
NS = B_FULL // N_CORES  # samples per core


def _ttiles(T, w=512):
    out, t0 = [], 0
    while t0 < T:
        out.append((t0, min(w, T - t0)))
        t0 += w
    return out


def build(ns=NS, T=T_MEL, TPH=T_PHON):
    """Build and compile the per-core Bass kernel (ns samples, seq len T)."""
    import concourse.bacc as bacc
    import concourse.tile as tile
    import concourse.mybir as mybir
    from concourse.alu_op_type import AluOpType as aop
    from concourse.masks import make_identity

    f32 = mybir.dt.float32
    bf16 = mybir.dt.bfloat16
    AF = mybir.ActivationFunctionType
    AX = mybir.AxisListType

    S = TPH + 1          # phoneme positions incl. prepended blank
    TP = T + 2           # zero border column each side
    SP = S + 2
    TT = _ttiles(T)          # 512-wide t tiles
    TBLK = _ttiles(T, 128)   # 128-wide t blocks
    SBLK = _ttiles(S, 128)   # s chunks
    EB = E // 128            # 3
    HB = 2 * E // 128        # 6
    VB = V // 128            # 2
    XMW = 516                # xm chunk slot width (512 + 2 halo + margin)

    nc = bacc.Bacc("TRN2", debug=False, target_bir_lowering=False)

    # ---------------- DRAM I/O ----------------
    d_mels = nc.dram_tensor("mels", [ns, MEL_DIMS, T], bf16, kind="ExternalInput").ap()
    d_embph = nc.dram_tensor("embph", [ns, E, SP], bf16, kind="ExternalInput").ap()
    d_melmask = nc.dram_tensor("melmask", [ns, TP], bf16, kind="ExternalInput").ap()
    d_phmask = nc.dram_tensor("phmask", [ns, SP], bf16, kind="ExternalInput").ap()
    d_phpen = nc.dram_tensor("phpen", [ns, S], f32, kind="ExternalInput").ap()

    d_meproj = nc.dram_tensor("me_proj", [MEL_DIMS, E], bf16, kind="ExternalInput").ap()
    d_mew = nc.dram_tensor("me_w", [4, 3, E, 2 * E], bf16, kind="ExternalInput").ap()
    d_pew = nc.dram_tensor("pe_w", [4, 3, E, 2 * E], bf16, kind="ExternalInput").ap()
    d_pdw = nc.dram_tensor("pd_w", [2, 3, 2 * E, 4 * E], bf16, kind="ExternalInput").ap()
    d_mdw = nc.dram_tensor("md_w", [2, 3, DEC_H, 2 * DEC_H], bf16, kind="ExternalInput").ap()
    d_pdlin = nc.dram_tensor("pd_lin", [2 * E, V], bf16, kind="ExternalInput").ap()
    d_mdproj = nc.dram_tensor("md_proj", [V, DEC_H], bf16, kind="ExternalInput").ap()
    d_mdlin = nc.dram_tensor("md_lin", [DEC_H, MEL_DIMS], bf16, kind="ExternalInput").ap()
    d_pdlinb_row = nc.dram_tensor("pd_lin_b_row", [1, V], bf16, kind="ExternalInput").ap()
    d_mdlinb_row = nc.dram_tensor("md_lin_b_row", [1, MEL_DIMS], bf16, kind="ExternalInput").ap()

    d_meprojb = nc.dram_tensor("me_proj_b", [E], f32, kind="ExternalInput").ap()
    d_meb = nc.dram_tensor("me_b", [4, 2 * E], f32, kind="ExternalInput").ap()
    d_peb = nc.dram_tensor("pe_b", [4, 2 * E], f32, kind="ExternalInput").ap()
    d_pdb = nc.dram_tensor("pd_b", [2, 4 * E], f32, kind="ExternalInput").ap()
    d_mdb = nc.dram_tensor("md_b", [2, 2 * DEC_H], f32, kind="ExternalInput").ap()
    d_mdprojb = nc.dram_tensor("md_proj_b", [DEC_H], f32, kind="ExternalInput").ap()
    d_pdlinb = nc.dram_tensor("pd_lin_b", [V], f32, kind="ExternalInput").ap()

    d_out = nc.dram_tensor("out", [ns, T, V + MEL_DIMS], f32, kind="ExternalOutput").ap()

    with tile.TileContext(nc) as tc:
        cpool = tc.alloc_tile_pool(name="consts", bufs=1)
        wring = tc.alloc_tile_pool(name="wring", bufs=54)
        apool = tc.alloc_tile_pool(name="acts", bufs=1)
        xring = tc.alloc_tile_pool(name="xmring", bufs=14)
        spool = tc.alloc_tile_pool(name="scratch", bufs=2)
        pspool = tc.alloc_tile_pool(name="psum", bufs=6, space="PSUM")

        def ps_tile(name="ps"):
            t = pspool.tile([128, 512], f32, tag="ps", name=name)
            return t

        def ps_tile_bf(name="psb"):
            # PE transpose writes through in the input dtype
            t = pspool.tile([128, 512], bf16, tag="ps", name=name)
            return t

        # ---------------- constants / resident weights ----------------
        ones_row = cpool.tile([1, 128], bf16, name="ones_row")
        nc.vector.memset(ones_row, 1.0)
        ones_col = cpool.tile([128, 1], bf16, name="ones_col")
        nc.vector.memset(ones_col, 1.0)
        ident = cpool.tile([128, 128], bf16, name="ident")
        make_identity(nc, ident)

        meproj_sb = cpool.tile([MEL_DIMS, E], bf16, name="meproj_sb")
        nc.sync.dma_start(meproj_sb, d_meproj)
        pdlin_sb = cpool.tile([128, HB, V], bf16, name="pdlin_sb")
        for c in range(HB):
            nc.sync.dma_start(pdlin_sb[:, c, :], d_pdlin[c * 128:(c + 1) * 128, :])
        mdproj_sb = cpool.tile([128, VB, DEC_H], bf16, name="mdproj_sb")
        for c in range(VB):
            nc.sync.dma_start(mdproj_sb[:, c, :], d_mdproj[c * 128:(c + 1) * 128, :])
        mdlin_sb = cpool.tile([128, MEL_DIMS], bf16, name="mdlin_sb")
        nc.sync.dma_start(mdlin_sb, d_mdlin)
        pdlinb_row = cpool.tile([1, V], bf16, name="pdlinb_row")
        nc.sync.dma_start(pdlinb_row, d_pdlinb_row)
        mdlinb_row = cpool.tile([1, MEL_DIMS], bf16, name="mdlinb_row")
        nc.sync.dma_start(mdlinb_row, d_mdlinb_row)

        # per-partition bias tables: [128, nlayer, nblocks]
        b_meproj = cpool.tile([128, EB], f32, name="b_meproj")
        nc.sync.dma_start(b_meproj, d_meprojb.rearrange("(a p) -> p a", p=128))
        b_me = cpool.tile([128, 4, HB], f32, name="b_me")
        nc.sync.dma_start(b_me, d_meb.rearrange("l (a p) -> p l a", p=128))
        b_pe = cpool.tile([128, 4, HB], f32, name="b_pe")
        nc.sync.dma_start(b_pe, d_peb.rearrange("l (a p) -> p l a", p=128))
        b_pd = cpool.tile([128, 2, 2 * HB], f32, name="b_pd")
        nc.sync.dma_start(b_pd, d_pdb.rearrange("l (a p) -> p l a", p=128))
        b_md = cpool.tile([128, 2, 2], f32, name="b_md")
        nc.sync.dma_start(b_md, d_mdb.rearrange("l (a p) -> p l a", p=128))
        b_mdproj = cpool.tile([128, 1], f32, name="b_mdproj")
        nc.sync.dma_start(b_mdproj, d_mdprojb.rearrange("(a p) -> p a", p=128))
        b_pdlin = cpool.tile([128, VB], f32, name="b_pdlin")
        nc.sync.dma_start(b_pdlin, d_pdlinb.rearrange("(a p) -> p a", p=128))

        # ---------------- persistent activations (per-sample reuse) ------
        decx = [apool.tile([128, TP], bf16, name=f"decx{c}") for c in range(HB)]
        phx = [apool.tile([128, SP], bf16, name=f"phx{c}") for c in range(EB)]
        ph2 = [apool.tile([128, S], bf16, name=f"ph2_{c}") for c in range(EB)]
        phT = [apool.tile([128, E], bf16, name=f"phT{i}") for i in range(len(SBLK))]
        mdx = [apool.tile([128, TP], bf16, name="mdx0")]
        logitbf = [apool.tile([128, TP], bf16, name=f"logitbf{v}") for v in range(VB)]
        pen_sb = apool.tile([1, S], bf16, name="pen_sb")

        # zero border columns (masks have zero borders so xm chunks inherit
        # zeros, but x borders are read when building xm chunks)
        for t_ in decx + phx + mdx:
            w = t_.shape[1]
            nc.vector.memset(t_[:, 0:1], 0.0)
            nc.vector.memset(t_[:, w - 1:w], 0.0)

        # ---------------- helpers ----------------
        def load_conv_weights(dram, l, n_cin, cout_w, name):
            """Stream one conv layer's weights through the ring.

            Returns dict (k, c, j) -> tile of [128, min(768, cout_w)] where j
            indexes 768-wide chunks of the cout dim.
            """
            njc = (cout_w + 767) // 768
            cw = min(768, cout_w)
            tiles = {}
            for k in range(3):
                for c in range(n_cin):
                    for j in range(njc):
                        wt = wring.tile([128, cw], bf16, tag="wconv",
                                        name=f"{name}{l}_{k}_{c}_{j}")
                        nc.sync.dma_start(
                            wt, dram[l, k, c * 128:(c + 1) * 128,
                                     j * cw:(j + 1) * cw])
                        tiles[(k, c, j)] = wt
            return tiles

        def glu_layer(x_tiles, n_cin, tiles_list, TPAD, mask_rep, wa, wg, bias_a, bias_g):
            """One masked GLU conv block, in place on x_tiles (bf16).

            wa(k, c, h) / wg(k, c, h) -> lhsT AP [128, 128] for the a/g couts.
            bias_a(h)/bias_g(h) -> [128, 1] fp32 AP.
            Residual uses the masked input (matches reference semantics).
            """
            n_half = n_cin  # cout == 2*cin for every GLU here
            xm = {}

            def emit_xm(i):
                t0, W = tiles_list[i]
                cw = min(W + 2, TPAD - t0)
                for c in range(n_cin):
                    xt = xring.tile([128, XMW], bf16, tag="xm", name=f"xm{c}_{i}")
                    nc.vector.tensor_tensor(
                        out=xt[:, :cw], in0=x_tiles[c][:, t0:t0 + cw],
                        in1=mask_rep[:, t0:t0 + cw], op=aop.mult)
                    xm[(c, i)] = xt

            emit_xm(0)
            for i, (t0, W) in enumerate(tiles_list):
                if i + 1 < len(tiles_list):
                    emit_xm(i + 1)
                for h in range(n_half):
                    a_ps = ps_tile("a_ps")
                    g_ps = ps_tile("g_ps")
                    nmm = 3 * n_cin
                    for half, ps in ((0, a_ps), (1, g_ps)):
                        idx = 0
                        for k in range(3):
                            for c in range(n_cin):
                                lhsT = wa(k, c, h) if half == 0 else wg(k, c, h)
                                nc.tensor.matmul(
                                    ps[:, :W], lhsT,
                                    xm[(c, i)][:, k:k + W],
                                    start=(idx == 0), stop=(idx == nmm - 1))
                                idx += 1
                    sig = spool.tile([128, 512], f32, tag="sig", name="sig")
                    nc.scalar.activation(sig[:, :W], g_ps[:, :W], AF.Sigmoid,
                                         bias=bias_g(h), scale=1.0)
                    tmp = spool.tile([128, 512], f32, tag="tmp", name="tmp")
                    nc.vector.scalar_tensor_tensor(
                        out=tmp[:, :W], in0=a_ps[:, :W], scalar=bias_a(h),
                        in1=sig[:, :W], op0=aop.add, op1=aop.mult)
                    # x = tmp + xm   (masked residual, exact in bf16)
                    nc.vector.tensor_tensor(
                        out=x_tiles[h][:, 1 + t0:1 + t0 + W], in0=tmp[:, :W],
                        in1=xm[(h, i)][:, 1:1 + W], op=aop.add)

        import concourse.bass as bass

        def bcast_row(row_ap):
            # [1, N] AP -> stride-0 partition broadcast AP [128, N] for DMA
            return bass.AP(tensor=row_ap.tensor, offset=row_ap.offset,
                           ap=[[0, 128]] + [list(d) for d in row_ap.ap[1:]])

        # ================ per-sample pipeline ================
        for s in range(ns):
            # ---- masks (partition-broadcast via stride-0 DMA) ----
            melmask = spool.tile([128, TP], bf16, tag="melmask", name="melmask")
            nc.sync.dma_start(out=melmask, in_=bcast_row(d_melmask[s:s + 1, :]))
            phmask = spool.tile([128, SP], bf16, tag="phmask", name="phmask")
            nc.sync.dma_start(out=phmask, in_=bcast_row(d_phmask[s:s + 1, :]))

            # ---- mel projection ----
            mels_sb = spool.tile([MEL_DIMS, T], bf16, tag="mels", name="mels_sb")
            nc.sync.dma_start(mels_sb, d_mels[s])
            for (t0, W) in TT:
                for eb in range(EB):
                    ps = ps_tile("proj_ps")
                    nc.tensor.matmul(ps[:, :W],
                                     meproj_sb[:, eb * 128:(eb + 1) * 128],
                                     mels_sb[:, t0:t0 + W], start=True, stop=True)
                    nc.scalar.activation(decx[eb][:, 1 + t0:1 + t0 + W], ps[:, :W],
                                         AF.Identity, bias=b_meproj[:, eb:eb + 1],
                                         scale=1.0)

            # ---- mel encoder: 4 GLU layers on decx[0:3] ----
            for l in range(4):
                wt = load_conv_weights(d_mew, l, EB, 2 * E, "mew")
                glu_layer(decx[:EB], EB, TT, TP, melmask,
                          wa=lambda k, c, h, wt=wt: wt[(k, c, 0)][:, h * 128:(h + 1) * 128],
                          wg=lambda k, c, h, wt=wt: wt[(k, c, 0)][:, (EB + h) * 128:(EB + h + 1) * 128],
                          bias_a=lambda h, l=l: b_me[:, l, h:h + 1],
                          bias_g=lambda h, l=l: b_me[:, l, EB + h:EB + h + 1])

            # ---- phoneme embedding + encoder ----
            for c in range(EB):
                nc.sync.dma_start(phx[c], d_embph[s, c * 128:(c + 1) * 128, :])
            for l in range(4):
                wt = load_conv_weights(d_pew, l, EB, 2 * E, "pew")
                glu_layer(phx, EB, [(0, S)], SP, phmask,
                          wa=lambda k, c, h, wt=wt: wt[(k, c, 0)][:, h * 128:(h + 1) * 128],
                          wg=lambda k, c, h, wt=wt: wt[(k, c, 0)][:, (EB + h) * 128:(EB + h + 1) * 128],
                          bias_a=lambda h, l=l: b_pe[:, l, h:h + 1],
                          bias_g=lambda h, l=l: b_pe[:, l, EB + h:EB + h + 1])

            # ---- attention prep: ph2 = 2*ph_enc, p2 = sum(ph^2), phT ----
            p2_ps = ps_tile("p2_ps")
            for c in range(EB):
                nc.vector.tensor_scalar_mul(ph2[c][:, :S], phx[c][:, 1:1 + S], 2.0)
                sq = spool.tile([128, S], bf16, tag="sq", name="sq")
                nc.vector.tensor_tensor(sq[:, :S], phx[c][:, 1:1 + S],
                                        phx[c][:, 1:1 + S], op=aop.mult)
                nc.tensor.matmul(p2_ps[0:1, :S], ones_col, sq[:, :S],
                                 start=(c == 0), stop=(c == EB - 1))
            phpen_f = spool.tile([1, S], f32, tag="phpen", name="phpen_f")
            nc.sync.dma_start(phpen_f, d_phpen[s:s + 1, :])
            # pen = -p2 + phpen   (phpen is 0 valid / -1e9 masked)
            nc.vector.scalar_tensor_tensor(
                out=pen_sb[0:1, :S], in0=p2_ps[0:1, :S], scalar=-1.0,
                in1=phpen_f[0:1, :S], op0=aop.mult, op1=aop.add)
            for c in range(EB):
                for si, (s0, sw) in enumerate(SBLK):
                    tr = ps_tile_bf("trph_ps")
                    nc.tensor.transpose(tr[:sw, :128],
                                        phx[c][:, 1 + s0:1 + s0 + sw],
                                        ident)
                    nc.scalar.copy(phT[si][:sw, c * 128:(c + 1) * 128],
                                   tr[:sw, :128])

            # ---- attention: softmax over phonemes, context -> decx[3:6] ----
            for (t0, TW) in TBLK:
                s_ps = ps_tile("s_ps")
                for c in range(EB):
                    nc.tensor.matmul(s_ps[:TW, :S],
                                     decx[c][:, 1 + t0:1 + t0 + TW],
                                     ph2[c][:, :S], start=(c == 0), stop=False)
                nc.tensor.matmul(s_ps[:TW, :S], ones_row[:, :TW], pen_sb[:, :S],
                                 start=False, stop=True)
                negmx = spool.tile([128, 1], f32, tag="negmx", bufs=3, name="negmx")
                nc.vector.reduce_max(negmx[:TW], s_ps[:TW, :S], axis=AX.X,
                                     negate=True)
                attn = spool.tile([128, S], bf16, tag="attn", bufs=3, name="attn")
                sumexp = spool.tile([128, 1], f32, tag="sumexp", bufs=3, name="sumexp")
                nc.scalar.activation(attn[:TW, :S], s_ps[:TW, :S], AF.Exp,
                                     bias=negmx[:TW], scale=1.0,
                                     accum_out=sumexp[:TW])
                rcp = spool.tile([128, 1], f32, tag="rcp", bufs=3, name="rcp")
                nc.vector.reciprocal(rcp[:TW], sumexp[:TW])
                nc.vector.tensor_scalar_mul(attn[:TW, :S], attn[:TW, :S], rcp[:TW])
                atT = []
                for si, (s0, sw) in enumerate(SBLK):
                    tr = ps_tile_bf("trat_ps")
                    nc.tensor.transpose(tr[:sw, :TW], attn[:TW, s0:s0 + sw],
                                        ident[:TW, :TW])
                    at = spool.tile([128, 128], bf16, tag=f"attnT{si}", name=f"attnT{si}")
                    nc.scalar.copy(at[:sw, :TW], tr[:sw, :TW])
                    atT.append(at)
                for eb in range(EB):
                    ctx = ps_tile("ctx_ps")
                    for si, (s0, sw) in enumerate(SBLK):
                        nc.tensor.matmul(ctx[:, :TW],
                                         phT[si][:sw, eb * 128:(eb + 1) * 128],
                                         atT[si][:sw, :TW],
                                         start=(si == 0), stop=(si == len(SBLK) - 1))
                    nc.vector.tensor_copy(decx[EB + eb][:, 1 + t0:1 + t0 + TW],
                                          ctx[:, :TW])

            # ---- decoder: 2 GLU layers on decx[0:6] ----
            for l in range(2):
                wt = load_conv_weights(d_pdw, l, HB, 4 * E, "pdw")
                glu_layer(decx, HB, TT, TP, melmask,
                          wa=lambda k, c, h, wt=wt: wt[(k, c, 0)][:, h * 128:(h + 1) * 128],
                          wg=lambda k, c, h, wt=wt: wt[(k, c, 1)][:, h * 128:(h + 1) * 128],
                          bias_a=lambda h, l=l: b_pd[:, l, h:h + 1],
                          bias_g=lambda h, l=l: b_pd[:, l, HB + h:HB + h + 1])

            # ---- logits (feature-major) -> logitbf, then mel_h0 ----
            for (t0, W) in TT:
                for vb in range(VB):
                    lf = ps_tile("lf_ps")
                    for c in range(HB):
                        nc.tensor.matmul(lf[:, :W],
                                         pdlin_sb[:, c, vb * 128:(vb + 1) * 128],
                                         decx[c][:, 1 + t0:1 + t0 + W],
                                         start=(c == 0), stop=(c == HB - 1))
                    nc.scalar.activation(logitbf[vb][:, 1 + t0:1 + t0 + W],
                                         lf[:, :W], AF.Identity,
                                         bias=b_pdlin[:, vb:vb + 1], scale=1.0)
                mh = ps_tile("mh_ps")
                for vb in range(VB):
                    nc.tensor.matmul(mh[:, :W], mdproj_sb[:, vb, :],
                                     logitbf[vb][:, 1 + t0:1 + t0 + W],
                                     start=(vb == 0), stop=(vb == VB - 1))
                nc.scalar.activation(mdx[0][:, 1 + t0:1 + t0 + W], mh[:, :W],
                                     AF.Identity, bias=b_mdproj[:, 0:1], scale=1.0)

            # ---- mel decoder: 2 GLU layers on mdx ----
            for l in range(2):
                wt = load_conv_weights(d_mdw, l, 1, 2 * DEC_H, "mdw")
                glu_layer(mdx, 1, TT, TP, melmask,
                          wa=lambda k, c, h, wt=wt: wt[(k, c, 0)][:, h * 128:(h + 1) * 128],
                          wg=lambda k, c, h, wt=wt: wt[(k, c, 0)][:, (1 + h) * 128:(2 + h) * 128],
                          bias_a=lambda h, l=l: b_md[:, l, h:h + 1],
                          bias_g=lambda h, l=l: b_md[:, l, 1 + h:2 + h])

            # ---- log_softmax (token-major) + mel preds + output ----
            for (t0, TW) in TBLK:
                lg = ps_tile("lg_ps")
                for c in range(HB):
                    nc.tensor.matmul(lg[:TW, :V], decx[c][:, 1 + t0:1 + t0 + TW],
                                     pdlin_sb[:, c, :], start=(c == 0), stop=False)
                nc.tensor.matmul(lg[:TW, :V], ones_row[:, :TW], pdlinb_row,
                                 start=False, stop=True)
                negmx2 = spool.tile([128, 1], f32, tag="negmx2", bufs=3, name="negmx2")
                nc.vector.reduce_max(negmx2[:TW], lg[:TW, :V], axis=AX.X,
                                     negate=True)
                esc = spool.tile([128, V], bf16, tag="esc", name="esc")
                se2 = spool.tile([128, 1], f32, tag="se2", bufs=3, name="se2")
                nc.scalar.activation(esc[:TW, :V], lg[:TW, :V], AF.Exp,
                                     bias=negmx2[:TW], scale=1.0,
                                     accum_out=se2[:TW])
                lse = spool.tile([128, 1], f32, tag="lse", bufs=3, name="lse")
                nc.scalar.activation(lse[:TW], se2[:TW], AF.Ln)
                nsh = spool.tile([128, 1], f32, tag="nsh", bufs=3, name="nsh")
                nc.vector.tensor_tensor(nsh[:TW], negmx2[:TW], lse[:TW],
                                        op=aop.subtract)
                outlp = spool.tile([128, V], f32, tag="outlp", bufs=3, name="outlp")
                nc.vector.tensor_scalar_add(outlp[:TW, :V], lg[:TW, :V], nsh[:TW])
                nc.sync.dma_start(d_out[s, t0:t0 + TW, 0:V], outlp[:TW, :V])

                mp = ps_tile("mp_ps")
                nc.tensor.matmul(mp[:TW, :MEL_DIMS], mdx[0][:, 1 + t0:1 + t0 + TW],
                                 mdlin_sb[:, :MEL_DIMS], start=True, stop=False)
                nc.tensor.matmul(mp[:TW, :MEL_DIMS], ones_row[:, :TW], mdlinb_row,
                                 start=False, stop=True)
                outmp = spool.tile([128, MEL_DIMS], f32, tag="outmp", bufs=3, name="outmp")
                nc.scalar.activation(outmp[:TW, :MEL_DIMS], mp[:TW, :MEL_DIMS],
                                     AF.Tanh)
                nc.sync.dma_start(d_out[s, t0:t0 + TW, V:V + MEL_DIMS],
                                  outmp[:TW, :MEL_DIMS])

        pspool.release()
        spool.release()
        xring.release()
        apool.release()
        wring.release()
        cpool.release()

    nc.compile()
    return nc


def preprocess(inputs, ns=NS, T=T_MEL, TPH=T_PHON, n_cores=N_CORES):
    """Host-side prep: transpose/pad/cast, build masks, shard per core."""
    S = TPH + 1
    TP = T + 2
    SP = S + 2
    B = ns * n_cores

    mels = np.asarray(inputs['mels'], np.float32)[:B, :T]
    phonemes = np.asarray(inputs['phonemes']).astype(np.int64)[:B, :TPH]
    mel_lens = np.asarray(inputs['mel_lens']).astype(np.int64)[:B]
    phoneme_lens = np.asarray(inputs['phoneme_lens']).astype(np.int64)[:B]
    emb = np.asarray(inputs['emb'], np.float32)

    mels_t = np.ascontiguousarray(mels.transpose(0, 2, 1)).astype(BF)  # [B,80,T]

    ph = np.concatenate([np.zeros((B, 1), np.int64), phonemes], axis=1)  # [B,S]
    embph = emb[ph]                                    # [B, S, E] f32
    embph_t = np.zeros((B, E, SP), np.float32)
    embph_t[:, :, 1:1 + S] = embph.transpose(0, 2, 1)
    embph_t = embph_t.astype(BF)

    t_idx = np.arange(T)
    melmask = np.zeros((B, TP), np.float32)
    melmask[:, 1:1 + T] = (t_idx[None, :] < mel_lens[:, None]).astype(np.float32)
    melmask = melmask.astype(BF)

    s_idx = np.arange(S)
    ph_valid = s_idx[None, :] <= phoneme_lens[:, None]
    phmask = np.zeros((B, SP), np.float32)
    phmask[:, 1:1 + S] = ph_valid.astype(np.float32)
    phmask = phmask.astype(BF)
    phpen = np.where(ph_valid, 0.0, -1e9).astype(np.float32)  # [B, S]

    shared = {
        'me_proj': np.asarray(inputs['me_proj_W'], np.float32).astype(BF),
        'me_w': np.asarray(inputs['me_W'], np.float32).astype(BF),
        'pe_w': np.asarray(inputs['pe_W'], np.float32).astype(BF),
        'pd_w': np.asarray(inputs['pd_W'], np.float32).astype(BF),
        'md_w': np.asarray(inputs['md_W'], np.float32).astype(BF),
        'pd_lin': np.asarray(inputs['pd_lin_W'], np.float32).astype(BF),
        'md_proj': np.asarray(inputs['md_proj_W'], np.float32).astype(BF),
        'md_lin': np.asarray(inputs['md_lin_W'], np.float32).astype(BF),
        'pd_lin_b_row': np.asarray(inputs['pd_lin_b'], np.float32)[None, :].astype(BF),
        'md_lin_b_row': np.asarray(inputs['md_lin_b'], np.float32)[None, :].astype(BF),
        'me_proj_b': np.asarray(inputs['me_proj_b'], np.float32),
        'me_b': np.asarray(inputs['me_b'], np.float32),
        'pe_b': np.asarray(inputs['pe_b'], np.float32),
        'pd_b': np.asarray(inputs['pd_b'], np.float32),
        'md_b': np.asarray(inputs['md_b'], np.float32),
        'md_proj_b': np.asarray(inputs['md_proj_b'], np.float32),
        'pd_lin_b': np.asarray(inputs['pd_lin_b'], np.float32),
    }

    in_maps = []
    for core in range(n_cores):
        sl = slice(core * ns, (core + 1) * ns)
        m = dict(shared)
        m['mels'] = np.ascontiguousarray(mels_t[sl])
        m['embph'] = np.ascontiguousarray(embph_t[sl])
        m['melmask'] = np.ascontiguousarray(melmask[sl])
        m['phmask'] = np.ascontiguousarray(phmask[sl])
        m['phpen'] = np.ascontiguousarray(phpen[sl])
        in_maps.append(m)
    return in_maps


_CACHE = {}


def _get_nc():
    if 'nc' not in _CACHE:
        _CACHE['nc'] = build()
    return _CACHE['nc']


def kernel(**inputs) -> np.ndarray:
    from concourse.bass_utils import run_bass_kernel_spmd
    nc = _get_nc()
    in_maps = preprocess(inputs)
    res = run_bass_kernel_spmd(nc, in_maps, core_ids=list(range(N_CORES)))
    out = np.concatenate([r['out'] for r in res.results], axis=0)
    return np.ascontiguousarray(out.astype(np.float32))


if __name__ == '__main__':
    import reference
    inputs = reference.setup_inputs()
    inputs = {k: np.asarray(v) for k, v in inputs.items()}
    out = kernel(**inputs)
    print(out.shape, out.dtype)
